# revision 5
# baseline (speedup 1.0000x reference)
"""MoE layer (B=2,T=1024,D=2048,F=768,E=16,K=2) on 8 NeuronCores.

Expert-parallel: 16 experts sorted by routed-token count; the 8 heaviest go
in slot 0 (capacity C0), the 8 lightest in slot 1 (capacity C1 <= C0), one
of each per core. Host computes the router (softmax -> top-2 -> renormalize,
~0.3% of FLOPs), gathers each expert's tokens into fixed-capacity transposed
buffers, and the device kernel runs the sparse SwiGLU FFN in bf16 with f32
PSUM accumulation. The per-token routing weight is applied on the host
during the scatter-add, so no combine-weight tensor ships to the device.

The kernel is simultaneously at the HBM roofline (~21MB in / ~350GB/s) and
the PE roofline (~66us of matmul issue), so the schedule is built around
stream deadlines:
- tokens staged transposed (xgt [D, C] as [P, KD*C]) so gate/up produce
  hT [F, C] directly in the lhsT layout the down projection wants.
- the down projection is STAGED: three passes of 2 f-chunks each, partial
  yT accumulated in SBUF bf16. This defers the deadline of down-weight
  slab h to (down_start + h*T_down/3), which is what makes the total
  input stream (~60us of wire time) fit before the final pass.
- the ACT HWDGE ring starts ~2us earlier than the SP ring, so it carries
  the bootstrap-critical j0 tiles at sub-slab granularity (xt00 halves
  interleaved with wg-j0 halves) followed by the rest of e0's gate slabs
  and down slabs 1,2; the SP ring (whose first byte is absorbed by a tiny
  warm-up DMA) carries xt01 + e0's up slabs, then e1's full weight set
  (the sync engine is idle all kernel, so its triggers are free). gpsimd
  SWDGE: down slab 0 of each expert + y(e0) out.
- per-j (0.5MB) slabs for e0 keep the early PE j-loop fed at the ring's
  ~0.2MB/us; the first real matmul starts ~t+5us instead of ~t+13us.
- both experts' full weight sets are held in SBUF (~165KB/part).
- ~8 N=512 + 12 N=256 garbage matmuls at kernel start warm the PE clock
  (HAM) during the initial DMA ramp; the kernel-end tail is dominated by
  the fixed NEFF epilogue (a ~5.4us per-engine semaphore-reset ladder),
  so the last y1 chunks ship as small solo DMAs to minimize drain time.
"""

import numpy as np
from contextlib import ExitStack

import concourse.bass as bass
import concourse.tile as tile
from concourse import mybir
from concourse.bass_utils import run_bass_kernel_spmd

B, T, D, F, E, TOPK = 2, 1024, 2048, 768, 16, 2
NCORES = 8
EPC = E // NCORES  # experts per core (2 slots)
P = 128


def _split_waits(nc, max_waits=1):
    """walrus on this image rejects >1 sync-wait per instruction
    (setupSyncWait: "Too many sync wait commands"); split extras into
    preceding same-engine NoOps."""
    for f in nc.m.functions:
        for b in f.blocks:
            insts = b.instructions
            idx = 0
            while idx < len(insts):
                inst = insts[idx]
                si = getattr(inst, "sync_info", None)
                if si is not None and si.on_wait and len(si.on_wait) > max_waits:
                    waits = list(si.on_wait)
                    extra, keep = waits[:-max_waits], waits[-max_waits:]
                    pos = idx
                    for j in range(0, len(extra), max_waits):
                        chunk = extra[j : j + max_waits]
                        nop = mybir.InstNoOp(name=f"{inst.name}_ws{j}", ins=[], outs=[])
                        nop.engine = inst.engine
                        nop.sync_info = mybir.SyncInfo(on_wait=chunk, on_update=[])
                        insts.insert(pos, nop)
                        pos += 1
                        idx += 1
                    inst.sync_info = mybir.SyncInfo(
                        on_wait=keep, on_update=list(si.on_update)
                    )
                idx += 1


def build_moe(C0, C1):
    """Per-core kernel: slot 0 capacity C0, slot 1 capacity C1 (each %8==0)."""
    assert C0 % 8 == 0 and C1 % 8 == 0 and 128 <= C1 <= C0 <= 512
    CS = (C0, C1)
    KD = D // P  # 16 k-tiles over D
    KF = F // P  # 6 f-chunks over F
    MD = D // P  # 16 m-chunks over D (down proj, yT layout)
    XS = 4  # token DMA slabs per expert
    bf16 = mybir.dt.bfloat16
    f32 = mybir.dt.float32

    nc = bass.Bass("TRN2", target_bir_lowering=False, debug=False, num_devices=NCORES)
    # host pre-tiled layouts (>=2KB contiguous per partition per DMA):
    #   xgt{s}[p, k*C + c] = x_gathered[s, k*128+p, c]
    #   wg/wu[e, j, p, k*128+f] = w[e, k*128+p, j*128+f]     (slab per f-chunk j)
    xgt0 = nc.declare_dram_parameter("xgt0", [P, KD * C0], bf16, isOutput=False)
    xgt1 = nc.declare_dram_parameter("xgt1", [P, KD * C1], bf16, isOutput=False)
    wg = nc.declare_dram_parameter("wg", [EPC, KF, P, KD * P], bf16, isOutput=False)
    wu = nc.declare_dram_parameter("wu", [EPC, KF, P, KD * P], bf16, isOutput=False)
    wd = nc.declare_dram_parameter("wd", [EPC, F, D], bf16, isOutput=False)
    y0 = nc.declare_dram_parameter("y0", [D, C0], bf16, isOutput=True)
    y1 = nc.declare_dram_parameter("y1", [D, C1], bf16, isOutput=True)
    xgts = (xgt0, xgt1)
    ys = (y0, y1)

    with tile.TileContext(nc) as tc, ExitStack() as ctx:
        xp = ctx.enter_context(tc.tile_pool(name="xp", bufs=1))
        wgp = ctx.enter_context(tc.tile_pool(name="wgp", bufs=2))
        # dt tiles single-buffered: e1's down-weight DMA then naturally waits
        # for e0's pass-h reads to finish -- a free throttle that keeps that
        # traffic out of the oversubscribed pre-transition window.
        wdp = ctx.enter_context(tc.tile_pool(name="wdp", bufs=1))
        hp = ctx.enter_context(tc.tile_pool(name="hp", bufs=1))
        sp = ctx.enter_context(tc.tile_pool(name="sp", bufs=2))
        op = ctx.enter_context(tc.tile_pool(name="op", bufs=1))
        # one shared 8-bank PSUM ring: gate phase runs 4 j-iterations deep,
        # down phases run 8 accumulation groups ahead of the evictions.
        pp = ctx.enter_context(tc.tile_pool(name="pp", bufs=8, space="PSUM"))

        # PE warmup: garbage matmuls with no dependencies run during the
        # initial DMA ramp so HAM un-throttles (1.2->2.4GHz) before real work.
        # ~8 N=512 cold (3.4us, enough to trip the HAM SHORT window) then
        # N=256 fillers at warm rate bridge until the first slabs land ~t+5.
        wsb = sp.tile([P, 512], bf16, tag="warm_sb", bufs=1)
        nc.vector.memset(wsb[:], 0)
        for _ in range(8):
            wps = pp.tile([P, 512], f32, tag="ps")
            nc.tensor.matmul(wps[:], wsb[:, :P], wsb[:], start=True, stop=True)
        for _ in range(12):
            wps = pp.tile([P, 256], f32, tag="ps")
            nc.tensor.matmul(wps[:], wsb[:, :P], wsb[:, :256], start=True, stop=True)

        gts = [[None] * KF for _ in range(EPC)]  # (tile, idx-within-group)
        uts = [[None] * KF for _ in range(EPC)]
        dts = [[None] * 3 for _ in range(EPC)]
        xts = [[None] * 2, [None] * 2]  # 2 token halves per expert
        xns = (2, 2)

        def trig_w(e, j0, nj, eng, w, store, wtag):
            t = wgp.tile(
                [P, nj, KD * P], bf16, tag=f"{wtag}{e}g{j0}", name=f"{wtag}{e}g{j0}",
                bufs=1,
            )
            eng.dma_start(t[:], w[e, j0 : j0 + nj].rearrange("j p c -> p j c"))
            for i in range(nj):
                store[e][j0 + i] = (t, i)

        def trig_dt(e, h, eng):
            dt = wdp.tile([P, KF // 3, D], bf16, tag=f"dt{h}")
            eng.dma_start(
                dt[:],
                wd[e].rearrange("(k p) d -> p k d", p=P)[:, bass.ts(h, KF // 3), :],
            )
            dts[e][h] = dt

        def trig_xt(e, h, eng):
            C = CS[e]
            xt = xp.tile([P, (KD // 2) * C], bf16, tag=f"xt{e}_{h}")
            eng.dma_start(xt[:], xgts[e][:, bass.ts(h, (KD // 2) * C)])
            xts[e][h] = xt

        # Two HWDGE rings: the ACT ring (scalar) moves first bytes ~1.5us
        # after its trigger, the SP ring (sync) ~3.4us.  Each sustains
        # ~200GB/s, FIFO per ring, so per-ring ORDER = deadline order.
        #
        # scalar ring (fast start): the j0 critical path, interleaved at
        # sub-slab granularity so the first k-chain can start ~t+5, then
        # the remaining e0 gate slabs and e0 down slabs 1,2.
        # sync ring: a tiny warm-up DMA (absorbs the ring's slow first
        # byte), then xt half1 + all e0 up slabs, then e1's full weight
        # set (the sync engine is otherwise idle all kernel).
        wdm = sp.tile([P, 16], bf16, tag="warm_dma", bufs=1)
        nc.sync.dma_start(wdm[:], xgt0[:, :16])

        C = CS[0]
        xt0 = xp.tile([P, (KD // 2) * C], bf16, tag="xt0_0")
        xts[0][0] = xt0
        g00 = wgp.tile([P, 1, KD * P], bf16, tag="wg0g0", name="wg0g0", bufs=1)
        gts[0][0] = (g00, 0)
        HK = (KD // 4) * C  # 4 k-tiles of tokens
        nc.scalar.dma_start(xt0[:, :HK], xgt0[:, :HK])
        nc.scalar.dma_start(g00[:, 0, : KD * P // 2], wg[0, 0, :, : KD * P // 2])
        nc.scalar.dma_start(xt0[:, HK:], xgt0[:, HK : 2 * HK])
        nc.scalar.dma_start(g00[:, 0, KD * P // 2 :], wg[0, 0, :, KD * P // 2 :])

        trig_xt(0, 1, nc.sync)
        for j in range(KF):
            trig_w(0, j, 1, nc.sync, wu, uts, "wu")

        for j in range(1, KF):
            trig_w(0, j, 1, nc.scalar, wg, gts, "wg")
        trig_dt(0, 1, nc.scalar)
        trig_dt(0, 2, nc.scalar)
        trig_dt(0, 0, nc.gpsimd)

        # e1 weights bulk-queued on the idle sync ring in deadline order
        # (paired slabs; data arrives ~28-49us, e1 gate starts ~41us).
        trig_w(1, 0, 2, nc.sync, wg, gts, "wg")
        trig_w(1, 2, 2, nc.sync, wg, gts, "wg")
        trig_w(1, 4, 2, nc.sync, wg, gts, "wg")
        trig_w(1, 0, 2, nc.sync, wu, uts, "wu")
        trig_w(1, 2, 2, nc.sync, wu, uts, "wu")

        # remaining mid-kernel loads the scalar engine issues between its
        # pass-h0 eviction copies (ring backlog covers their deadlines)
        e0down_trigs = [("xt", 0), ("xt", 1), ("ut45", None)]

        def pop_trig():
            if e0down_trigs:
                kind, a = e0down_trigs.pop(0)
                if kind == "xt":
                    trig_xt(1, a, nc.scalar)
                else:
                    trig_w(1, 4, 2, nc.scalar, wu, uts, "wu")

        for e in range(EPC):
            C = CS[e]
            # ---- gate/up + SwiGLU -> hT [F, C] bf16 ----
            ht = hp.tile([P, KF, C], bf16, tag=f"ht{e}")
            for j in range(KF):
                gt, gi = gts[e][j]
                ut, ui = uts[e][j]
                g_ps = pp.tile([P, C], f32, tag="ps")
                u_ps = pp.tile([P, C], f32, tag="ps")
                kdn = KD // xns[e]
                for k in range(KD):
                    nc.tensor.matmul(
                        g_ps[:],
                        gt[:, gi, bass.ts(k, P)],
                        xts[e][k // kdn][:, bass.ts(k % kdn, C)],
                        start=(k == 0),
                        stop=(k == KD - 1),
                    )
                for k in range(KD):
                    nc.tensor.matmul(
                        u_ps[:],
                        ut[:, ui, bass.ts(k, P)],
                        xts[e][k // kdn][:, bass.ts(k % kdn, C)],
                        start=(k == 0),
                        stop=(k == KD - 1),
                    )
                sil = sp.tile([P, C], f32, tag="sil")
                nc.scalar.activation(
                    sil[:], g_ps[:], mybir.ActivationFunctionType.Silu
                )
                nc.vector.tensor_mul(ht[:, j, :], sil[:], u_ps[:])
            if e == 0:
                trig_dt(0, 2, nc.scalar)

            # ---- down proj: yT[m] = sum_h sum_{j in slab h} ----
            ydst = ys[e].rearrange("(m p) c -> p m c", p=P)
            ysb = op.tile([P, MD, C], bf16, tag=f"ysb{e}")
            if e == 0:
                # staged: 3 passes of 2 f-chunks, partials accumulated in
                # SBUF bf16 -- defers down-weight slab h's deadline to pass h,
                # which lets the input stream fit during the e0 phases.
                for h in range(3):
                    for m in range(MD):
                        y_ps = pp.tile([P, C], f32, tag="ps")
                        for i in range(2):
                            nc.tensor.matmul(
                                y_ps[:],
                                dts[e][h][:, i, bass.ts(m, P)],
                                ht[:, 2 * h + i, :],
                                start=(i == 0),
                                stop=(i == 1),
                            )
                        if h == 0:
                            # ACT engine owns the first partial (psum port)
                            nc.scalar.copy(ysb[:, m, :], y_ps[:])
                            if m % 2 == 1:
                                pop_trig()
                        else:
                            nc.vector.tensor_add(
                                ysb[:, m, :], ysb[:, m, :], y_ps[:]
                            )
                        if h == 2 and m % 4 == 3:
                            nc.gpsimd.dma_start(
                                ydst[:, m - 3 : m + 1, :], ysb[:, m - 3 : m + 1, :]
                            )
                    if h == 0:
                        while e0down_trigs:
                            pop_trig()
                        # e1 down slabs join the queues here; WAR-gated on
                        # e0's pass reads (wdp bufs=1) so they stream during
                        # e1's gate phase, not before. Slab 0 rides the
                        # otherwise-idle SWDGE queue.
                        trig_dt(1, 0, nc.gpsimd)
                        trig_dt(1, 1, nc.scalar)
                        trig_dt(1, 2, nc.sync)
            else:
                # input is all on-chip by now: two subrounds of 8 m-chunks,
                # full 6-tile accumulation in PSUM (one eviction per m-chunk,
                # split across DVE and ACT so neither gates the PE).
                psub = [None] * 8
                for m0 in range(0, MD, 8):
                    for h in range(3):
                        for m in range(m0, m0 + 8):
                            y_ps = psub[m - m0] if h else pp.tile(
                                [P, C], f32, tag="ps"
                            )
                            if h == 0:
                                psub[m - m0] = y_ps
                            for i in range(2):
                                nc.tensor.matmul(
                                    y_ps[:],
                                    dts[e][h][:, i, bass.ts(m, P)],
                                    ht[:, 2 * h + i, :],
                                    start=(h == 0 and i == 0),
                                    stop=(h == 2 and i == 1),
                                )
                            if h == 2:
                                ev = nc.vector.tensor_copy if m % 2 else nc.scalar.copy
                                ev(ysb[:, m, :], y_ps[:])
                                # drain pattern: pairs early, SOLO chunks for
                                # the last two m so the final post-matmul DMA
                                # is one 66KB transfer on an empty ring (the
                                # kernel-end barrier waits on its receipt).
                                if m < 14 and m % 2 == 1:
                                    yeng = nc.sync if (m // 2) % 2 == 0 else nc.scalar
                                    yeng.dma_start(
                                        ydst[:, m - 1 : m + 1, :],
                                        ysb[:, m - 1 : m + 1, :],
                                    )
                                elif m == 14:
                                    nc.scalar.dma_start(
                                        ydst[:, m : m + 1, :], ysb[:, m : m + 1, :]
                                    )
                                elif m == 15:
                                    nc.sync.dma_start(
                                        ydst[:, m : m + 1, :], ysb[:, m : m + 1, :]
                                    )

    _split_waits(nc)
    return nc


_CACHE = {}


def _get_nc(C0, C1):
    if (C0, C1) not in _CACHE:
        _CACHE[(C0, C1)] = build_moe(C0, C1)
    return _CACHE[(C0, C1)]


def _route(x, router_w):
    """Replicates the reference router in f32: softmax over expert scores,
    top-2, renormalize."""
    xf = x.reshape(-1, D).astype(np.float32)
    scores = xf @ router_w.astype(np.float32)
    m = scores.max(axis=-1, keepdims=True)
    ex = np.exp(scores - m)
    probs = ex / ex.sum(axis=-1, keepdims=True)
    idx = np.argsort(-probs, axis=-1, kind="stable")[:, :TOPK]
    wts = np.take_along_axis(probs, idx, axis=-1)
    wts = wts / wts.sum(axis=-1, keepdims=True)
    return idx.astype(np.int32), wts.astype(np.float32)


def _cap(n):
    return min(512, max(P, -(-n // 8) * 8))


def kernel(x, router_w, gate_w, up_w, down_w):
    import ml_dtypes

    bf = ml_dtypes.bfloat16

    x = np.asarray(x)
    in_dtype = x.dtype
    xf = x.reshape(-1, D).astype(np.float32)
    idx, wts = _route(x, np.asarray(router_w))

    # token lists per expert
    tok_ids = [None] * E
    tok_wts = [None] * E
    counts = np.zeros(E, dtype=np.int64)
    for e in range(E):
        sel = np.nonzero(idx == e)
        tok_ids[e] = sel[0].astype(np.int64)
        tok_wts[e] = wts[sel[0], sel[1]]
        counts[e] = len(tok_ids[e])

    # heaviest 8 experts -> slot 0 (capacity C0), lightest 8 -> slot 1 (C1)
    order = np.argsort(-counts, kind="stable")
    slot_exp = [(int(order[c]), int(order[8 + c])) for c in range(NCORES)]
    C0 = _cap(int(counts[order[0]]))
    C1 = _cap(int(counts[order[8]]))

    nc = _get_nc(C0, C1)

    KD, KF = D // P, F // P

    def tile_gateup(w):
        # [E, D, F] -> [E, KF, P, KD*P] with w_t[e,j,p,k*P+f] = w[e,k*P+p,j*P+f]
        w = np.asarray(w).astype(bf)
        w = w.reshape(E, KD, P, KF, P).transpose(0, 3, 2, 1, 4)
        return np.ascontiguousarray(w.reshape(E, KF, P, KD * P))

    g16 = tile_gateup(gate_w)
    u16 = tile_gateup(up_w)
    d16 = np.asarray(down_w).astype(bf)
    xT = np.ascontiguousarray(xf.T)  # [D, B*T] f32

    in_maps = []
    for c in range(NCORES):
        im = {}
        eids = slot_exp[c]
        for s, C in ((0, C0), (1, C1)):
            e = eids[s]
            n = int(counts[e])
            xg = np.zeros((P, KD, C), dtype=bf)
            gath = xT[:, tok_ids[e]]  # [D, n] f32
            xg[:, :, :n] = gath.astype(bf).reshape(KD, P, n).transpose(1, 0, 2)
            im[f"xgt{s}"] = xg.reshape(P, KD * C)
        im["wg"] = np.ascontiguousarray(g16[list(eids)])
        im["wu"] = np.ascontiguousarray(u16[list(eids)])
        im["wd"] = np.ascontiguousarray(d16[list(eids)])
        in_maps.append(im)

    res = run_bass_kernel_spmd(nc, in_maps, list(range(NCORES)))

    out = np.zeros((B * T, D), dtype=np.float32)
    for c in range(NCORES):
        for s in range(EPC):
            e = slot_exp[c][s]
            n = int(counts[e])
            yv = res.results[c][f"y{s}"]  # [D, C] bf16
            out[tok_ids[e]] += tok_wts[e][:, None] * yv[:, :n].astype(np.float32).T
    return out.reshape(B, T, D).astype(in_dtype)



# revision 9
# speedup vs baseline: 1.0323x; 1.0323x over previous
"""MoE layer (B=2,T=1024,D=2048,F=768,E=16,K=2) on 8 NeuronCores.

Expert-parallel: 16 experts sorted by routed-token count; the 8 heaviest go
in slot 0 (capacity C0), the 8 lightest in slot 1 (capacity C1 <= C0), one
of each per core. Host computes the router (softmax -> top-2 -> renormalize,
~0.3% of FLOPs), gathers each expert's tokens into fixed-capacity transposed
buffers, and the device kernel runs the sparse SwiGLU FFN in bf16 with f32
PSUM accumulation. The per-token routing weight is applied on the host
during the scatter-add, so no combine-weight tensor ships to the device.

The kernel sits at the ridge point: ~21MB of input at ~400GB/s aggregate
wire (~53us) vs ~68us of PE matmul issue. The schedule:
- tokens staged transposed (xgt [D, C] as [P, KD*C]) so gate/up produce
  hT [F, C] directly in the lhsT layout the down projection wants.
- gate+up weights are host-packed per j-chunk into one [2, P, KD*P] slab
  so each j ships as a single fused 1.05MB DMA (big transfers keep the
  rings at wire speed; the baseline's turnaround problem).
- the ACT HWDGE ring starts ~1.5us after its first trigger, the SP ring
  ~3.4us (absorbed by a tiny warm-up DMA). Bootstrap tiles (e0 j0 gate/up
  as separate 0.5MB slabs, e0 tokens as 4 quarter DMAs) alternate across
  rings in global deadline order so the first real matmul issues ~t+6us.
- e0 down projection runs in TWO full-PSUM passes (f-tiles 0-3, then 4-5)
  with copy-only evictions split ACT/DVE by m-parity; the partial merge
  (ysbA += ysbB, 16 DVE adds) is deferred into e1's gate phase where the
  DVE is otherwise idle, and y(e0) streams out on the SWDGE queue behind
  it. This keeps the down passes PE-dense (the 3-pass staged variant was
  eviction-bound: DVE 86% busy) while the down-weight slab deadlines stay
  loose (dt00/dt01 by pass A, dt02 by pass B).
- e1's full weight set streams on the SP ring during e0 compute; e1 down
  accumulates all 6 f-tiles in PSUM per m-chunk (16 copy-evictions only).
- ~8 N=512 + 10 N=256 garbage matmuls at kernel start warm the PE clock
  (HAM) during the DMA ramp. The kernel-end tail is dominated by the
  fixed NEFF epilogue (~5.4us per-engine semaphore-reset ladder); the
  last y1 chunks ship as small solo DMAs to minimize the drain ahead
  of it.
"""

import numpy as np
from contextlib import ExitStack

import concourse.bass as bass
import concourse.tile as tile
from concourse import mybir
from concourse.bass_utils import run_bass_kernel_spmd

B, T, D, F, E, TOPK = 2, 1024, 2048, 768, 16, 2
NCORES = 8
EPC = E // NCORES  # experts per core (2 slots)
P = 128


def _split_waits(nc, max_waits=1):
    """walrus on this image rejects >1 sync-wait per instruction
    (setupSyncWait: "Too many sync wait commands"); split extras into
    preceding same-engine NoOps."""
    for f in nc.m.functions:
        for b in f.blocks:
            insts = b.instructions
            idx = 0
            while idx < len(insts):
                inst = insts[idx]
                si = getattr(inst, "sync_info", None)
                if si is not None and si.on_wait and len(si.on_wait) > max_waits:
                    waits = list(si.on_wait)
                    extra, keep = waits[:-max_waits], waits[-max_waits:]
                    pos = idx
                    for j in range(0, len(extra), max_waits):
                        chunk = extra[j : j + max_waits]
                        nop = mybir.InstNoOp(name=f"{inst.name}_ws{j}", ins=[], outs=[])
                        nop.engine = inst.engine
                        nop.sync_info = mybir.SyncInfo(on_wait=chunk, on_update=[])
                        insts.insert(pos, nop)
                        pos += 1
                        idx += 1
                    inst.sync_info = mybir.SyncInfo(
                        on_wait=keep, on_update=list(si.on_update)
                    )
                idx += 1


def build_moe(C0, C1):
    """Per-core kernel: slot 0 capacity C0, slot 1 capacity C1 (each %8==0)."""
    assert C0 % 8 == 0 and C1 % 8 == 0 and 128 <= C1 <= C0 <= 512
    CS = (C0, C1)
    KD = D // P  # 16 k-tiles over D
    KF = F // P  # 6 f-chunks over F
    MD = D // P  # 16 m-chunks over D (down proj, yT layout)
    bf16 = mybir.dt.bfloat16
    f32 = mybir.dt.float32

    nc = bass.Bass("TRN2", target_bir_lowering=False, debug=False, num_devices=NCORES)
    # host pre-tiled layouts (>=2KB contiguous per partition per DMA):
    #   xgt{s}[p, k*C + c] = x_gathered[s, k*128+p, c]
    #   wgu[e, j, g, p, k*128+f] = w_g[e, k*128+p, j*128+f]  (g=0 gate, 1 up)
    xgt0 = nc.declare_dram_parameter("xgt0", [P, KD * C0], bf16, isOutput=False)
    xgt1 = nc.declare_dram_parameter("xgt1", [P, KD * C1], bf16, isOutput=False)
    wgu = nc.declare_dram_parameter("wgu", [EPC, KF, 2, P, KD * P], bf16, isOutput=False)
    wd = nc.declare_dram_parameter("wd", [EPC, F, D], bf16, isOutput=False)
    y0 = nc.declare_dram_parameter("y0", [D, C0], bf16, isOutput=True)
    y1 = nc.declare_dram_parameter("y1", [D, C1], bf16, isOutput=True)
    xgts = (xgt0, xgt1)
    ys = (y0, y1)

    with tile.TileContext(nc) as tc, ExitStack() as ctx:
        xp = ctx.enter_context(tc.tile_pool(name="xp", bufs=1))
        wgp = ctx.enter_context(tc.tile_pool(name="wgp", bufs=1))
        # dt tiles single-buffered: e1's down-weight DMA then naturally waits
        # for e0's pass reads to finish -- a free throttle that keeps that
        # traffic out of the oversubscribed early window.
        wdp = ctx.enter_context(tc.tile_pool(name="wdp", bufs=1))
        hp = ctx.enter_context(tc.tile_pool(name="hp", bufs=1))
        sp = ctx.enter_context(tc.tile_pool(name="sp", bufs=2))
        op = ctx.enter_context(tc.tile_pool(name="op", bufs=1))
        # one shared 8-bank PSUM ring.
        pp = ctx.enter_context(tc.tile_pool(name="pp", bufs=8, space="PSUM"))

        # PE warmup: garbage matmuls with no dependencies run during the
        # initial DMA ramp so HAM un-throttles (1.2->2.4GHz) before real work.
        wsb = sp.tile([P, 512], bf16, tag="warm_sb", bufs=1)
        nc.vector.memset(wsb[:], 0)
        for _ in range(8):
            wps = pp.tile([P, 512], f32, tag="ps")
            nc.tensor.matmul(wps[:], wsb[:, :P], wsb[:], start=True, stop=True)
        for _ in range(10):
            wps = pp.tile([P, 256], f32, tag="ps")
            nc.tensor.matmul(wps[:], wsb[:, :P], wsb[:, :256], start=True, stop=True)

        gts = [[None] * KF for _ in range(EPC)]  # (tile, idx) per j
        uts = [[None] * KF for _ in range(EPC)]
        dts = [[None] * 3 for _ in range(EPC)]
        xparts = [None, None]  # e0: 4 quarter tiles; e1: 1 full tile
        XKT = (4, KD)  # k-tiles per token tile

        def trig_gu(e, j, eng):
            t = wgp.tile(
                [P, 2, KD * P], bf16, tag=f"gu{e}j{j}", name=f"gu{e}j{j}", bufs=1
            )
            eng.dma_start(t[:], wgu[e, j].rearrange("g p c -> p g c"))
            gts[e][j] = (t, 0)
            uts[e][j] = (t, 1)

        def trig_dt(e, h, eng):
            dt = wdp.tile([P, KF // 3, D], bf16, tag=f"dt{h}")
            eng.dma_start(
                dt[:],
                wd[e].rearrange("(k p) d -> p k d", p=P)[:, bass.ts(h, KF // 3), :],
            )
            dts[e][h] = dt

        def xop(e, k):
            """(tile, column-slice) covering token k-tile k of expert e."""
            kt = XKT[e]
            return xparts[e][k // kt][:, bass.ts(k % kt, CS[e])]

        # Two HWDGE rings at ~200GB/s each under load, FIFO per ring; the
        # ACT ring's first byte beats the SP ring's by ~2us, so bootstrap
        # tiles alternate rings in global deadline order.  The sync engine
        # is idle all kernel, so the SP ring carries everything that would
        # otherwise cost ACT-engine issue slots.
        wdm = sp.tile([P, 16], bf16, tag="warm_dma", bufs=1)
        nc.sync.dma_start(wdm[:], xgt0[:, :16])

        # e0 j0 gate and up ship as separate 0.5MB tiles so the gate half
        # arrives (and unblocks the first matmuls) first; j>0 ships fused.
        g00 = wgp.tile([P, 1, KD * P], bf16, tag="g0j0", name="g0j0", bufs=1)
        gts[0][0] = (g00, 0)
        nc.scalar.dma_start(g00[:], wgu[0, 0, 0:1].rearrange("g p c -> p g c"))

        # e0 tokens: 4 quarter DMAs on the SP ring (separate tiles so each
        # quarter's consumers wait only for its own transfer).
        xparts[0] = []
        for q in range(4):
            xt = xp.tile([P, 4 * C0], bf16, tag=f"xt0q{q}")
            nc.sync.dma_start(xt[:], xgt0[:, bass.ts(q, 4 * C0)])
            xparts[0].append(xt)

        u00 = wgp.tile([P, 1, KD * P], bf16, tag="u0j0", name="u0j0", bufs=1)
        uts[0][0] = (u00, 0)
        nc.scalar.dma_start(u00[:], wgu[0, 0, 1:2].rearrange("g p c -> p g c"))
        trig_gu(0, 1, nc.scalar)
        trig_gu(0, 2, nc.sync)
        trig_gu(0, 3, nc.scalar)
        trig_gu(0, 4, nc.sync)
        trig_gu(0, 5, nc.scalar)
        trig_dt(0, 0, nc.sync)
        trig_dt(0, 1, nc.scalar)
        trig_dt(0, 2, nc.scalar)
        # e1 weights bulk-queued on the idle SP ring in deadline order
        trig_gu(1, 0, nc.sync)
        trig_gu(1, 1, nc.sync)
        trig_gu(1, 2, nc.sync)

        # mid-kernel loads the scalar engine issues between its pass-A
        # eviction copies (the ring backlog covers their deadlines)
        def trig_xt1():
            xt = xp.tile([P, KD * C1], bf16, tag="xt1")
            nc.scalar.dma_start(xt[:], xgt1[:, :])
            xparts[1] = [xt]

        e0down_trigs = [trig_xt1] + [
            (lambda j=j: trig_gu(1, j, nc.scalar)) for j in (3, 4, 5)
        ]

        def pop_trig():
            if e0down_trigs:
                e0down_trigs.pop(0)()

        for e in range(EPC):
            C = CS[e]
            # ---- gate/up + SwiGLU -> hT [F, C] bf16 ----
            ht = hp.tile([P, KF, C], bf16, tag=f"ht{e}")
            for j in range(KF):
                gt, gi = gts[e][j]
                ut, ui = uts[e][j]
                g_ps = pp.tile([P, C], f32, tag="ps")
                u_ps = pp.tile([P, C], f32, tag="ps")
                for k in range(KD):
                    nc.tensor.matmul(
                        g_ps[:],
                        gt[:, gi, bass.ts(k, P)],
                        xop(e, k),
                        start=(k == 0),
                        stop=(k == KD - 1),
                    )
                for k in range(KD):
                    nc.tensor.matmul(
                        u_ps[:],
                        ut[:, ui, bass.ts(k, P)],
                        xop(e, k),
                        start=(k == 0),
                        stop=(k == KD - 1),
                    )
                sil = sp.tile([P, C], f32, tag="sil")
                nc.scalar.activation(
                    sil[:], g_ps[:], mybir.ActivationFunctionType.Silu
                )
                nc.vector.tensor_mul(ht[:, j, :], sil[:], u_ps[:])
                if e == 1:
                    # deferred e0 partial merge + y0 drain ride the
                    # otherwise-idle DVE / SWDGE queue during e1's gate.
                    if j < 4:
                        m0 = 4 * j
                        nc.vector.tensor_add(
                            ysbA[:, m0 : m0 + 4, :],
                            ysbA[:, m0 : m0 + 4, :],
                            ysbB[:, m0 : m0 + 4, :],
                        )
                        nc.gpsimd.dma_start(
                            ys[0].rearrange("(m p) c -> p m c", p=P)[
                                :, m0 : m0 + 4, :
                            ],
                            ysbA[:, m0 : m0 + 4, :],
                        )

            # ---- down proj: yT[m] = sum_f dwT[f, m] @ hT[f, :] ----
            ydst = ys[e].rearrange("(m p) c -> p m c", p=P)
            if e == 0:
                # two full-PSUM passes (f-tiles 0-3, then 4-5): copy-only
                # evictions split ACT/DVE by m-parity keep the PE dense;
                # the A+B merge is deferred into e1's gate phase.
                ysbA = op.tile([P, MD, C], bf16, tag="ysbA")
                ysbB = op.tile([P, MD, C], bf16, tag="ysbB")
                for m in range(MD):
                    y_ps = pp.tile([P, C], f32, tag="ps")
                    for hi in range(4):
                        nc.tensor.matmul(
                            y_ps[:],
                            dts[0][hi // 2][:, hi % 2, bass.ts(m, P)],
                            ht[:, hi, :],
                            start=(hi == 0),
                            stop=(hi == 3),
                        )
                    ev = nc.vector.tensor_copy if m % 2 else nc.scalar.copy
                    ev(ysbA[:, m, :], y_ps[:])
                    if m % 4 == 3:
                        pop_trig()
                # e1 down slabs join the queues here; WAR-gated on e0's
                # pass reads (wdp bufs=1). Slab 0 rides the SWDGE queue.
                trig_dt(1, 0, nc.gpsimd)
                trig_dt(1, 1, nc.sync)
                trig_dt(1, 2, nc.sync)
                for m in range(MD):
                    y_ps = pp.tile([P, C], f32, tag="ps")
                    for i in range(2):
                        nc.tensor.matmul(
                            y_ps[:],
                            dts[0][2][:, i, bass.ts(m, P)],
                            ht[:, 4 + i, :],
                            start=(i == 0),
                            stop=(i == 1),
                        )
                    ev = nc.vector.tensor_copy if m % 2 else nc.scalar.copy
                    ev(ysbB[:, m, :], y_ps[:])
            else:
                # input is all on-chip: two subrounds of 8 m-chunks, full
                # 6-tile accumulation in PSUM, one eviction per m-chunk.
                ysb = op.tile([P, MD, C], bf16, tag="ysb1")
                psub = [None] * 8
                for m0 in range(0, MD, 8):
                    for h in range(3):
                        for m in range(m0, m0 + 8):
                            y_ps = psub[m - m0] if h else pp.tile(
                                [P, C], f32, tag="ps"
                            )
                            if h == 0:
                                psub[m - m0] = y_ps
                            for i in range(2):
                                nc.tensor.matmul(
                                    y_ps[:],
                                    dts[e][h][:, i, bass.ts(m, P)],
                                    ht[:, 2 * h + i, :],
                                    start=(h == 0 and i == 0),
                                    stop=(h == 2 and i == 1),
                                )
                            if h == 2:
                                ev = nc.vector.tensor_copy if m % 2 else nc.scalar.copy
                                ev(ysb[:, m, :], y_ps[:])
                                # pairs early, SOLO chunks for the last two
                                # m so the final post-matmul DMA is one
                                # small transfer on an empty ring (the
                                # kernel-end barrier waits on its receipt).
                                if m < 14 and m % 2 == 1:
                                    yeng = nc.sync if (m // 2) % 2 == 0 else nc.scalar
                                    yeng.dma_start(
                                        ydst[:, m - 1 : m + 1, :],
                                        ysb[:, m - 1 : m + 1, :],
                                    )
                                elif m == 14:
                                    nc.scalar.dma_start(
                                        ydst[:, m : m + 1, :], ysb[:, m : m + 1, :]
                                    )
                                elif m == 15:
                                    nc.sync.dma_start(
                                        ydst[:, m : m + 1, :], ysb[:, m : m + 1, :]
                                    )

    _split_waits(nc)
    return nc


_CACHE = {}


def _get_nc(C0, C1):
    if (C0, C1) not in _CACHE:
        _CACHE[(C0, C1)] = build_moe(C0, C1)
    return _CACHE[(C0, C1)]


def _route(x, router_w):
    """Replicates the reference router in f32: softmax over expert scores,
    top-2, renormalize."""
    xf = x.reshape(-1, D).astype(np.float32)
    scores = xf @ router_w.astype(np.float32)
    m = scores.max(axis=-1, keepdims=True)
    ex = np.exp(scores - m)
    probs = ex / ex.sum(axis=-1, keepdims=True)
    idx = np.argsort(-probs, axis=-1, kind="stable")[:, :TOPK]
    wts = np.take_along_axis(probs, idx, axis=-1)
    wts = wts / wts.sum(axis=-1, keepdims=True)
    return idx.astype(np.int32), wts.astype(np.float32)


def _cap(n):
    return min(512, max(P, -(-n // 8) * 8))


def kernel(x, router_w, gate_w, up_w, down_w):
    import ml_dtypes

    bf = ml_dtypes.bfloat16

    x = np.asarray(x)
    in_dtype = x.dtype
    xf = x.reshape(-1, D).astype(np.float32)
    idx, wts = _route(x, np.asarray(router_w))

    # token lists per expert
    tok_ids = [None] * E
    tok_wts = [None] * E
    counts = np.zeros(E, dtype=np.int64)
    for e in range(E):
        sel = np.nonzero(idx == e)
        tok_ids[e] = sel[0].astype(np.int64)
        tok_wts[e] = wts[sel[0], sel[1]]
        counts[e] = len(tok_ids[e])

    # heaviest 8 experts -> slot 0 (capacity C0), lightest 8 -> slot 1 (C1)
    order = np.argsort(-counts, kind="stable")
    slot_exp = [(int(order[c]), int(order[8 + c])) for c in range(NCORES)]
    C0 = _cap(int(counts[order[0]]))
    C1 = _cap(int(counts[order[8]]))

    nc = _get_nc(C0, C1)

    KD, KF = D // P, F // P

    def tile_gateup(w):
        # [E, D, F] -> [E, KF, P, KD*P] with w_t[e,j,p,k*P+f] = w[e,k*P+p,j*P+f]
        w = np.asarray(w).astype(bf)
        w = w.reshape(E, KD, P, KF, P).transpose(0, 3, 2, 1, 4)
        return np.ascontiguousarray(w.reshape(E, KF, P, KD * P))

    g16 = tile_gateup(gate_w)
    u16 = tile_gateup(up_w)
    # fused per-j gate/up slab: [E, KF, 2, P, KD*P]
    gu16 = np.ascontiguousarray(np.stack([g16, u16], axis=2))
    d16 = np.asarray(down_w).astype(bf)
    xT = np.ascontiguousarray(xf.T)  # [D, B*T] f32

    in_maps = []
    for c in range(NCORES):
        im = {}
        eids = slot_exp[c]
        for s, C in ((0, C0), (1, C1)):
            e = eids[s]
            n = int(counts[e])
            xg = np.zeros((P, KD, C), dtype=bf)
            gath = xT[:, tok_ids[e]]  # [D, n] f32
            xg[:, :, :n] = gath.astype(bf).reshape(KD, P, n).transpose(1, 0, 2)
            im[f"xgt{s}"] = xg.reshape(P, KD * C)
        im["wgu"] = np.ascontiguousarray(gu16[list(eids)])
        im["wd"] = np.ascontiguousarray(d16[list(eids)])
        in_maps.append(im)

    res = run_bass_kernel_spmd(nc, in_maps, list(range(NCORES)))

    out = np.zeros((B * T, D), dtype=np.float32)
    for c in range(NCORES):
        for s in range(EPC):
            e = slot_exp[c][s]
            n = int(counts[e])
            yv = res.results[c][f"y{s}"]  # [D, C] bf16
            out[tok_ids[e]] += tok_wts[e][:, None] * yv[:, :n].astype(np.float32).T
    return out.reshape(B, T, D).astype(in_dtype)
